# revision 11
# baseline (speedup 1.0000x reference)
import sys
import functools
import numpy as np

sys.path.insert(0, "/opt/trn_rl_repo")

from contextlib import ExitStack
from concourse import bass, bacc, mybir
import concourse.tile as tile
from concourse import bass_utils
from concourse.masks import make_identity

P_TOT = 20000   # points
DEG = 16        # neighbors per point
CIN = 16
COUT = 16
NCORES = 8
PTS_CORE = P_TOT // NCORES       # 2500 points per core
PARTS = 128
KPP = 20                         # points per partition (128*20 = 2560 slots)
PTS_PAD = PARTS * KPP            # 2560
NPP = KPP * DEG                  # 320 edges per partition
XI = 3 * CIN                     # 48
ELEM = 64                        # f32 per padded feature row (256B)
KC = 4                           # points per chunk
NCHUNK = KPP // KC               # 5
GCH = KC * DEG                   # 64 edges per partition per chunk
NIDX = GCH * PARTS               # 8192 gathered rows per chunk
SCH = NIDX // 16                 # 512 idx columns per chunk
GSUB = 64                        # rows per partition per dma_gather call
NSUB = GCH // GSUB               # sub-calls per chunk
NIDX_SUB = GSUB * PARTS          # idxs per call
SSUB = NIDX_SUB // 16            # idx columns per sub-call
SINGLE_PACKET = False            # True caps payload at 16KB/engine = 1024 idxs


@functools.lru_cache(maxsize=1)
def _build():
    nc = bacc.Bacc("TRN2", target_bir_lowering=False, debug=False,
                   num_swdge_queues=4)
    f32 = mybir.dt.float32
    i16 = mybir.dt.int16

    featp_d = nc.dram_tensor("featp", [P_TOT, ELEM], f32, kind="ExternalInput")
    gidx_d = nc.dram_tensor("gidx", [PARTS, NCHUNK * SCH], i16,
                            kind="ExternalInput")
    rad_d = nc.dram_tensor("rad", [PARTS, NPP * 3], f32, kind="ExternalInput")
    w_d = nc.dram_tensor("w2v", [XI, COUT], f32, kind="ExternalInput")
    out_d = nc.dram_tensor("out", [PARTS, NPP], f32, kind="ExternalOutput")

    with ExitStack() as ctx:
        tc = ctx.enter_context(tile.TileContext(nc))
        sb = ctx.enter_context(tc.tile_pool(name="sb", bufs=1))
        fpool = ctx.enter_context(tc.tile_pool(name="fpool", bufs=2))
        rfpool = ctx.enter_context(tc.tile_pool(name="rfpool", bufs=2))
        stpool = ctx.enter_context(tc.tile_pool(name="stpool", bufs=2))
        ps = ctx.enter_context(tc.tile_pool(name="ps", bufs=2, space="PSUM"))
        ps2 = ctx.enter_context(tc.tile_pool(name="ps2", bufs=2, space="PSUM"))

        gidx_t = sb.tile([PARTS, NCHUNK * SCH], dtype=i16)
        rad_t = sb.tile([PARTS, NPP * 3], dtype=f32)
        w_t = sb.tile([XI, COUT], dtype=f32)
        s_t = sb.tile([PARTS, KPP * XI], dtype=f32)
        out_t = sb.tile([PARTS, NPP], dtype=f32)
        ident = sb.tile([PARTS, PARTS], dtype=f32)

        make_identity(nc, ident[:])
        nc.sync.dma_start(out=gidx_t[:], in_=gidx_d[:])
        nc.sync.dma_start(out=rad_t[:], in_=rad_d[:])
        nc.sync.dma_start(out=w_t[:], in_=w_d[:])

        for ck in range(NCHUNK):
            f_t = fpool.tile([PARTS, GCH * ELEM], dtype=f32)
            for sc in range(NSUB):
                f_sub = f_t[:, sc * GSUB * ELEM:(sc + 1) * GSUB * ELEM]
                nc.gpsimd.dma_gather(
                    out_ap=f_sub.rearrange("p (g e) -> p g e", e=ELEM),
                    in_ap=featp_d[:],
                    idxs_ap=gidx_t[:, ck * SCH + sc * SSUB:
                                   ck * SCH + (sc + 1) * SSUB],
                    num_idxs=NIDX_SUB,
                    num_idxs_reg=NIDX_SUB,
                    elem_size=ELEM,
                    single_packet=SINGLE_PACKET,
                )

            rf_t = rfpool.tile([PARTS, GCH * XI], dtype=f32)
            f_v = (f_t[:].rearrange("p (g e) -> p g e", e=ELEM)[:, :, 0:CIN])
            rad_v = (rad_t[:, ck * GCH * 3:(ck + 1) * GCH * 3]
                     .rearrange("p (n x) -> p n x", x=3))
            rf_v = rf_t[:].rearrange("p (n c) -> p n c", c=XI)
            for x in range(3):
                nc.vector.tensor_tensor(
                    out=rf_v[:, :, x * CIN:(x + 1) * CIN],
                    in0=f_v,
                    in1=rad_v[:, :, x:x + 1].broadcast_to([PARTS, GCH, CIN]),
                    op=mybir.AluOpType.mult)

            for kk in range(KC):
                red_in = (rf_t[:, kk * DEG * XI:(kk + 1) * DEG * XI]
                          .rearrange("p (j c) -> p c j", c=XI))
                k = ck * KC + kk
                red_out = s_t[:, k * XI:(k + 1) * XI].unsqueeze(2)
                nc.vector.tensor_reduce(
                    out=red_out, in_=red_in,
                    axis=mybir.AxisListType.X, op=mybir.AluOpType.add)

            for kk in range(KC):
                k = ck * KC + kk
                st_ps = ps.tile([XI, PARTS], dtype=f32, space="PSUM")
                nc.tensor.transpose(
                    out=st_ps[:], in_=s_t[:, k * XI:(k + 1) * XI],
                    identity=ident[:])
                st_sb = stpool.tile([XI, PARTS], dtype=f32)
                nc.vector.tensor_copy(st_sb[:], st_ps[:])
                o_ps = ps2.tile([PARTS, COUT], dtype=f32, space="PSUM")
                nc.tensor.matmul(
                    out=o_ps[:], lhsT=st_sb[:], rhs=w_t[:],
                    start=True, stop=True)
                nc.vector.tensor_copy(out_t[:, k * COUT:(k + 1) * COUT], o_ps[:])

        nc.sync.dma_start(out=out_d[:], in_=out_t[:])

    nc.finalize()
    return nc


def _prep(inputs):
    features = np.asarray(inputs["features"], dtype=np.float32)
    radii = np.asarray(inputs["radii"], dtype=np.float32)
    bs = np.asarray(inputs["bs_slice"])
    Wk = np.asarray(inputs["Wk"], dtype=np.float32)
    n_norm = float(np.asarray(inputs["n_norm"]))

    featp = np.zeros((P_TOT, ELEM), np.float32)
    featp[:, :CIN] = features

    nbr = np.ascontiguousarray(bs[:, 1:1 + DEG]).astype(np.int64)  # [P_TOT, DEG]
    rad3 = radii.reshape(P_TOT, DEG, 3)
    w2v = np.ascontiguousarray(
        Wk.transpose(0, 2, 1).reshape(XI, COUT) / np.sqrt(n_norm)
    ).astype(np.float32)

    in_maps = []
    for c in range(NCORES):
        lo = c * PTS_CORE
        hi = lo + PTS_CORE
        idx_pad = np.zeros((PTS_PAD, DEG), np.int64)
        idx_pad[:PTS_CORE] = nbr[lo:hi]
        arr = idx_pad.reshape(PARTS, NPP)            # [128, g=k*16+j]
        stream = arr.T.ravel()                       # stream[g*128+p] = arr[p,g]
        tile16 = stream.reshape(-1, 16).T            # [16, NCHUNK*SCH]
        gidx = np.tile(tile16, (PARTS // 16, 1)).astype(np.int16)

        rad_pad = np.zeros((PTS_PAD, DEG, 3), np.float32)
        rad_pad[:PTS_CORE] = rad3[lo:hi]
        in_maps.append({
            "featp": featp,
            "gidx": np.ascontiguousarray(gidx),
            "rad": np.ascontiguousarray(rad_pad.reshape(PARTS, NPP * 3)),
            "w2v": w2v,
        })
    return in_maps


def run(inputs, trace=False):
    nc = _build()
    in_maps = _prep(inputs)
    r = bass_utils.run_bass_kernel_spmd(
        nc, in_maps, core_ids=list(range(NCORES)), trace=trace)
    outs = []
    for c in range(NCORES):
        o = np.asarray(r.results[c]["out"]).reshape(PTS_PAD, COUT)[:PTS_CORE]
        outs.append(o)
    full = np.ascontiguousarray(np.concatenate(outs, axis=0), dtype=np.float32)
    return full, r


def kernel(**inputs):
    out, _ = run(inputs, trace=False)
    return out


# revision 12
# speedup vs baseline: 1.4207x; 1.4207x over previous
import sys
import functools
import numpy as np

sys.path.insert(0, "/opt/trn_rl_repo")

from contextlib import ExitStack
from concourse import bass, bacc, mybir
import concourse.tile as tile
from concourse import bass_utils
from concourse.masks import make_identity

P_TOT = 20000   # points
DEG = 16        # neighbors per point
CIN = 16
COUT = 16
NCORES = 8
PTS_CORE = P_TOT // NCORES       # 2500 points per core
PARTS = 128
KPP = 20                         # points per partition (128*20 = 2560 slots)
PTS_PAD = PARTS * KPP            # 2560
NPP = KPP * DEG                  # 320 edges per partition
XI = 3 * CIN                     # 48
ELEM = 64                        # f32 per padded feature row (256B)
KC = 4                           # points per chunk
NCHUNK = KPP // KC               # 5
GCH = KC * DEG                   # 64 edges per partition per chunk
NIDX = GCH * PARTS               # 8192 gathered rows per chunk
SCH = NIDX // 16                 # 512 idx columns per chunk
GSUB = 64                        # rows per partition per dma_gather call
NSUB = GCH // GSUB               # sub-calls per chunk
NIDX_SUB = GSUB * PARTS          # idxs per call
SSUB = NIDX_SUB // 16            # idx columns per sub-call
SINGLE_PACKET = False            # True caps payload at 16KB/engine = 1024 idxs


@functools.lru_cache(maxsize=1)
def _build():
    nc = bacc.Bacc("TRN2", target_bir_lowering=False, debug=False,
                   num_swdge_queues=4)
    f32 = mybir.dt.float32
    i16 = mybir.dt.int16

    featp_d = nc.dram_tensor("featp", [P_TOT, ELEM], f32, kind="ExternalInput")
    gidx_d = nc.dram_tensor("gidx", [PARTS, NCHUNK * SCH], i16,
                            kind="ExternalInput")
    rad_d = nc.dram_tensor("rad", [PARTS, NPP * 3], f32, kind="ExternalInput")
    w_d = nc.dram_tensor("w2v", [XI, COUT], f32, kind="ExternalInput")
    out_d = nc.dram_tensor("out", [PARTS, NPP], f32, kind="ExternalOutput")

    with ExitStack() as ctx:
        tc = ctx.enter_context(tile.TileContext(nc))
        sb = ctx.enter_context(tc.tile_pool(name="sb", bufs=1))
        fpool = ctx.enter_context(tc.tile_pool(name="fpool", bufs=2))
        rfpool = ctx.enter_context(tc.tile_pool(name="rfpool", bufs=2))
        stpool = ctx.enter_context(tc.tile_pool(name="stpool", bufs=2))
        ps = ctx.enter_context(tc.tile_pool(name="ps", bufs=2, space="PSUM"))
        ps2 = ctx.enter_context(tc.tile_pool(name="ps2", bufs=2, space="PSUM"))

        gidx_t = sb.tile([PARTS, NCHUNK * SCH], dtype=i16)
        rad_t = sb.tile([PARTS, NPP * 3], dtype=f32)
        w_t = sb.tile([XI, COUT], dtype=f32)
        s_t = sb.tile([PARTS, KPP * XI], dtype=f32)
        out_t = sb.tile([PARTS, NPP], dtype=f32)
        ident = sb.tile([PARTS, PARTS], dtype=f32)

        make_identity(nc, ident[:])
        nc.sync.dma_start(out=gidx_t[:], in_=gidx_d[:])
        nc.sync.dma_start(out=rad_t[:], in_=rad_d[:])
        nc.sync.dma_start(out=w_t[:], in_=w_d[:])

        for ck in range(NCHUNK):
            f_t = fpool.tile([PARTS, GCH * ELEM], dtype=f32)
            for sc in range(NSUB):
                f_sub = f_t[:, sc * GSUB * ELEM:(sc + 1) * GSUB * ELEM]
                nc.gpsimd.dma_gather(
                    out_ap=f_sub.rearrange("p (g e) -> p g e", e=ELEM),
                    in_ap=featp_d[:],
                    idxs_ap=gidx_t[:, ck * SCH + sc * SSUB:
                                   ck * SCH + (sc + 1) * SSUB],
                    num_idxs=NIDX_SUB,
                    num_idxs_reg=NIDX_SUB,
                    elem_size=ELEM,
                    single_packet=SINGLE_PACKET,
                    queue_num=(ck * NSUB + sc) % 4,
                )

            rf_t = rfpool.tile([PARTS, GCH * XI], dtype=f32)
            f_v = (f_t[:].rearrange("p (g e) -> p g e", e=ELEM)[:, :, 0:CIN])
            rad_v = (rad_t[:, ck * GCH * 3:(ck + 1) * GCH * 3]
                     .rearrange("p (n x) -> p n x", x=3))
            rf_v = rf_t[:].rearrange("p (n c) -> p n c", c=XI)
            for x in range(3):
                nc.vector.tensor_tensor(
                    out=rf_v[:, :, x * CIN:(x + 1) * CIN],
                    in0=f_v,
                    in1=rad_v[:, :, x:x + 1].broadcast_to([PARTS, GCH, CIN]),
                    op=mybir.AluOpType.mult)

            for kk in range(KC):
                red_in = (rf_t[:, kk * DEG * XI:(kk + 1) * DEG * XI]
                          .rearrange("p (j c) -> p c j", c=XI))
                k = ck * KC + kk
                red_out = s_t[:, k * XI:(k + 1) * XI].unsqueeze(2)
                nc.vector.tensor_reduce(
                    out=red_out, in_=red_in,
                    axis=mybir.AxisListType.X, op=mybir.AluOpType.add)

            for kk in range(KC):
                k = ck * KC + kk
                st_ps = ps.tile([XI, PARTS], dtype=f32, space="PSUM")
                nc.tensor.transpose(
                    out=st_ps[:], in_=s_t[:, k * XI:(k + 1) * XI],
                    identity=ident[:])
                st_sb = stpool.tile([XI, PARTS], dtype=f32)
                nc.vector.tensor_copy(st_sb[:], st_ps[:])
                o_ps = ps2.tile([PARTS, COUT], dtype=f32, space="PSUM")
                nc.tensor.matmul(
                    out=o_ps[:], lhsT=st_sb[:], rhs=w_t[:],
                    start=True, stop=True)
                nc.vector.tensor_copy(out_t[:, k * COUT:(k + 1) * COUT], o_ps[:])

        nc.sync.dma_start(out=out_d[:], in_=out_t[:])

    nc.finalize()
    return nc


def _prep(inputs):
    features = np.asarray(inputs["features"], dtype=np.float32)
    radii = np.asarray(inputs["radii"], dtype=np.float32)
    bs = np.asarray(inputs["bs_slice"])
    Wk = np.asarray(inputs["Wk"], dtype=np.float32)
    n_norm = float(np.asarray(inputs["n_norm"]))

    featp = np.zeros((P_TOT, ELEM), np.float32)
    featp[:, :CIN] = features

    nbr = np.ascontiguousarray(bs[:, 1:1 + DEG]).astype(np.int64)  # [P_TOT, DEG]
    rad3 = radii.reshape(P_TOT, DEG, 3)
    w2v = np.ascontiguousarray(
        Wk.transpose(0, 2, 1).reshape(XI, COUT) / np.sqrt(n_norm)
    ).astype(np.float32)

    in_maps = []
    for c in range(NCORES):
        lo = c * PTS_CORE
        hi = lo + PTS_CORE
        idx_pad = np.zeros((PTS_PAD, DEG), np.int64)
        idx_pad[:PTS_CORE] = nbr[lo:hi]
        arr = idx_pad.reshape(PARTS, NPP)            # [128, g=k*16+j]
        stream = arr.T.ravel()                       # stream[g*128+p] = arr[p,g]
        tile16 = stream.reshape(-1, 16).T            # [16, NCHUNK*SCH]
        gidx = np.tile(tile16, (PARTS // 16, 1)).astype(np.int16)

        rad_pad = np.zeros((PTS_PAD, DEG, 3), np.float32)
        rad_pad[:PTS_CORE] = rad3[lo:hi]
        in_maps.append({
            "featp": featp,
            "gidx": np.ascontiguousarray(gidx),
            "rad": np.ascontiguousarray(rad_pad.reshape(PARTS, NPP * 3)),
            "w2v": w2v,
        })
    return in_maps


def run(inputs, trace=False):
    nc = _build()
    in_maps = _prep(inputs)
    r = bass_utils.run_bass_kernel_spmd(
        nc, in_maps, core_ids=list(range(NCORES)), trace=trace)
    outs = []
    for c in range(NCORES):
        o = np.asarray(r.results[c]["out"]).reshape(PTS_PAD, COUT)[:PTS_CORE]
        outs.append(o)
    full = np.ascontiguousarray(np.concatenate(outs, axis=0), dtype=np.float32)
    return full, r


def kernel(**inputs):
    out, _ = run(inputs, trace=False)
    return out
